# revision 49
# baseline (speedup 1.0000x reference)
"""Multi-head cross-attention (B=32, N=128, M=2048, 16 heads x 64) on 8 TRN2 cores.

Strategy: pure data-parallel over the batch dim (4 batches/core). All matmul
operands are fp16 (fp32 accumulation in PSUM); softmax skips the max-subtraction
(scores are ~N(0,1), exp stays well inside fp16 range).

All host inputs are pre-shuffled into [128-partition, free] layouts so every
DMA is a contiguous per-partition copy; the two HWDGE queues (SP + ACT) load
in parallel, and ~5us of throwaway matmuls ramp the PE clock during the
initial DMA wait.

Per-core device program (b = 4 batches):
  batch 0 prologue ordered so PE work starts as soon as Wk + the first kv
  m-chunk land: all 8 khT chunks of b0 (m-chunk-outer, so each arriving kv
  chunk unlocks 32 matmuls), then qhT (Wq/qT stream in behind), then vh b0,
  then the b0 scores loop.
  qhT  = Wq^T q^T            (heads on partitions, scale 1/8 folded into q)
  per batch:
    vh   = kv Wv             (kv tokens on partitions, + ones column per head)
    khT_c = Wk^T kv^T        (per inner chunk c of 128 = head pair)
    scoresT = khT_c^T qhT    (K=128 with the other head's q half zeroed)
    attnT = exp(scoresT)     (ACT, fp16)
    av_h  = attnT_h^T [vh_h | 1]   ([n=128, 65]; col 64 = softmax denominator)
    out[n, h*64:+64] = av_h[:, 0:64] * recip(av_h[:, 64])  (per-partition scalar)
    outT_c = PE-transpose of out[:, c*128:+128]
    y = outT^T Wo + bo       (K=128 per chunk, accumulated over 8 chunks)
  The out-projection for batch b is deferred past batch b+1's vh so its tail
  hides under PE work; the last batch overlaps out-projection chunks 0-5 with
  the final AV pair.
"""
import numpy as np

NCORES = 8
B, BPC = 32, 4
N, M = 128, 2048
H, D = 16, 64
QD, KVD, INNER = 1024, 512, 1024

_cached = {}


def _build_nc():
    from contextlib import ExitStack

    import concourse.tile as tile
    from concourse import bacc, mybir
    from concourse.masks import make_identity

    F16 = mybir.dt.float16
    F32 = mybir.dt.float32
    AF = mybir.ActivationFunctionType

    nc = bacc.Bacc("TRN2", target_bir_lowering=False, debug=False,
                   num_devices=NCORES)
    # All inputs pre-shuffled on the host into [128-partition, free]
    # layouts so every DMA is a contiguous per-partition copy.
    qT_d = nc.dram_tensor("qT", [128, 8 * BPC * N], F16, kind="ExternalInput").ap()
    kvT_d = nc.dram_tensor("kvT", [BPC, 128, 4 * M], F16, kind="ExternalInput").ap()
    wq_d = nc.dram_tensor("Wq", [128, 8 * INNER], F16, kind="ExternalInput").ap()
    wk_d = nc.dram_tensor("Wk", [128, 4 * INNER], F16, kind="ExternalInput").ap()
    wv_d = nc.dram_tensor("Wv", [128, 4 * INNER], F16, kind="ExternalInput").ap()
    wo_d = nc.dram_tensor("Wo", [128, 8 * QD], F16, kind="ExternalInput").ap()
    bo_d = nc.dram_tensor("bo", [128, QD], F32, kind="ExternalInput").ap()
    y_d = nc.dram_tensor("y", [BPC, N, QD], F32, kind="ExternalOutput").ap()

    with tile.TileContext(nc) as tc, ExitStack() as ctx:
        const = ctx.enter_context(tc.tile_pool(name="const", bufs=1))
        kvt_pool = ctx.enter_context(tc.tile_pool(name="kvt", bufs=2))
        kht_pool = ctx.enter_context(tc.tile_pool(name="kht", bufs=8))
        vh_pool = ctx.enter_context(tc.tile_pool(name="vh", bufs=1))
        attn_pool = ctx.enter_context(tc.tile_pool(name="attn", bufs=18))
        out_pool = ctx.enter_context(tc.tile_pool(name="outp", bufs=2))
        outt_pool = ctx.enter_context(tc.tile_pool(name="outt", bufs=2))
        y_pool = ctx.enter_context(tc.tile_pool(name="yp", bufs=1))
        r_pool = ctx.enter_context(tc.tile_pool(name="rp", bufs=4))
        pp = ctx.enter_context(tc.tile_pool(name="pp", bufs=3, space="PSUM"))
        scp = ctx.enter_context(tc.tile_pool(name="scp", bufs=3, space="PSUM"))
        avp = ctx.enter_context(tc.tile_pool(name="avp", bufs=2, space="PSUM"))

        # ---- constants into SBUF, ordered so batch-0 khT can start first.
        # SP DGE: kvT[0] m-chunks, then Wo, bo.
        # ACT DGE in parallel: Wk (first), then Wq, qT, Wv.
        # wk_sb free layout: (c, k, ni) -> c*512 + k*128 + ni
        kvt0_sb = kvt_pool.tile([128, 4 * M], F16, tag="kvt")
        for mc in range(4):
            nc.sync.dma_start(kvt0_sb[:, mc * 2048:(mc + 1) * 2048],
                              kvT_d[0][:, mc * 2048:(mc + 1) * 2048])
        wk_sb = const.tile([128, 4 * INNER], F16)
        nc.scalar.dma_start(wk_sb[:], wk_d[:])
        wq_sb = const.tile([128, 8 * INNER], F16)
        nc.scalar.dma_start(wq_sb[:], wq_d[:])
        qt_sb = const.tile([128, 8 * BPC * N], F16)
        nc.scalar.dma_start(qt_sb[:], qT_d[:])
        wv_sb = const.tile([128, 4 * INNER], F16)
        nc.scalar.dma_start(wv_sb[:], wv_d[:])
        wo_sb = const.tile([128, 8 * QD], F16)
        nc.sync.dma_start(wo_sb[:], wo_d[:])
        bo_bc = const.tile([128, QD], F32)
        nc.sync.dma_start(bo_bc[:], bo_d[:])
        ident = const.tile([128, 128], F16)
        make_identity(nc, ident[:])

        # PE warm-up during the initial DMA wait: ~5us of throwaway matmuls
        # ramp the HAM clock so the first real matmuls run at full speed.
        warm_sb = const.tile([128, 512], F16)
        nc.vector.memset(warm_sb[:], 0.0)
        wp = pp.tile([128, 512], F32, tag="pp")
        for w in range(24):
            nc.tensor.matmul(wp[:], warm_sb[:, 0:128], warm_sb[:],
                             start=(w == 0), stop=(w == 23))

        BN = BPC * N  # 512

        # ---- khT for one batch, chunk c (head pair): [128, M] in SBUF
        def make_kht(kvt_sb, c):
            kht_sb = kht_pool.tile([128, M], F16, tag="kht")
            for n in range(4):
                p = pp.tile([128, 512], F32, tag="pp")
                for k in range(4):
                    nc.tensor.matmul(
                        p[:],
                        wk_sb[:, c * 512 + k * 128:c * 512 + (k + 1) * 128],
                        kvt_sb[:, n * 2048 + k * 512:n * 2048 + (k + 1) * 512],
                        start=(k == 0), stop=(k == 3),
                    )
                nc.vector.tensor_copy(kht_sb[:, n * 512:(n + 1) * 512], p[:])
            return kht_sb

        # ---- batch 0: all khT chunks first (only needs Wk + kvT[0]).
        # m-chunk-outer so each arriving kvT chunk unlocks 32 matmuls.
        kht0 = [kht_pool.tile([128, M], F16, tag="kht", name=f"kht0_{c}")
                for c in range(8)]
        for n in range(4):
            for c in range(8):
                p = pp.tile([128, 512], F32, tag="pp")
                for k in range(4):
                    nc.tensor.matmul(
                        p[:],
                        wk_sb[:, c * 512 + k * 128:c * 512 + (k + 1) * 128],
                        kvt0_sb[:, n * 2048 + k * 512:n * 2048 + (k + 1) * 512],
                        start=(k == 0), stop=(k == 3),
                    )
                nc.vector.tensor_copy(kht0[c][:, n * 512:(n + 1) * 512], p[:])

        # ---- qhT projection: all 4 batches at once, chunk c = head pair.
        # Layout (c, b, hi, n): per (c, b) the two head-halves sit in adjacent
        # 128-col blocks, each with the complementary 64 partition rows zeroed,
        # so ONE K=128 N=256 scores matmul serves both heads of the pair.
        qh2 = const.tile([128, 8 * 2 * BN], F16)
        qh2v = qh2[:].rearrange("p (c b i n) -> p c b i n", c=8, b=BPC, i=2)
        nc.vector.memset(qh2v[64:128, :, :, 0, :], 0.0)
        nc.vector.memset(qh2v[0:64, :, :, 1, :], 0.0)
        for c in range(8):
            p = pp.tile([128, BN], F32, tag="pp")
            for k in range(8):
                nc.tensor.matmul(
                    p[:],
                    wq_sb[:, k * INNER + c * 128:k * INNER + (c + 1) * 128],
                    qt_sb[:, k * BN:(k + 1) * BN],
                    start=(k == 0), stop=(k == 7),
                )
            nc.vector.tensor_copy(
                qh2v[0:64, c, :, 0, :],
                p[0:64, :].rearrange("p (b n) -> p b n", b=BPC),
            )
            nc.vector.tensor_copy(
                qh2v[64:128, c, :, 1, :],
                p[64:128, :].rearrange("p (b n) -> p b n", b=BPC),
            )

        vh_sb = vh_pool.tile([128, 16 * H * 65], F16, tag="vh")
        vh4 = vh_sb[:].rearrange("p (t h d) -> p t h d", h=H, d=65)
        nc.vector.memset(vh4[:, :, :, 64:65], 1.0)

        # vh = kv @ Wv for one batch (kv tokens on partitions). Evictions
        # alternate DVE/ACT: ACT is idle during this phase, and draining the
        # casts on both engines keeps DVE from entering the scores loop with
        # a backlog that delays the out_sb evictions the transposes wait on.
        def make_vh(kvt_sb):
            for t in range(16):
                for n2 in range(2):
                    p = pp.tile([128, 512], F32, tag="pp")
                    for k in range(4):
                        nc.tensor.matmul(
                            p[:],
                            kvt_sb[:, (t // 4) * 2048 + k * 512 + (t % 4) * 128:
                                   (t // 4) * 2048 + k * 512 + (t % 4) * 128 + 128],
                            wv_sb[:, k * INNER + n2 * 512:k * INNER + (n2 + 1) * 512],
                            start=(k == 0), stop=(k == 3),
                        )
                    dst = vh4[:, t, n2 * 8:(n2 + 1) * 8, 0:64]
                    src = p[:].rearrange("p (h d) -> p h d", d=64)
                    if t % 2 == 0:
                        nc.vector.tensor_copy(dst, src)
                    else:
                        nc.scalar.copy(dst, src)

        # Out-projection for a finished batch: K=128 accumulate over the 8
        # transposed chunks, add bias, store. Deferred until after the NEXT
        # batch's vh matmuls so its tail hides under PE work.
        def do_outproj(work):
            bb, out_prev, outT = work
            # transposes of the last two chunks, deferred past the next
            # batch's vh so the batch-end eviction latency is hidden
            for cc in (6, 7):
                ptr = avp.tile([128, 194], F32, tag="avp")
                pt = ptr[:, 130:194].bitcast(F16)
                nc.tensor.transpose(
                    pt, out_prev[:, cc * 128:(cc + 1) * 128], ident[:])
                nc.scalar.copy(outT[:, cc * 128:(cc + 1) * 128], pt)
            y_sb = y_pool.tile([128, QD], F32, tag="yp")
            yv = y_d[bb].rearrange("n (h q) -> n h q", h=2)
            for n2 in range(2):
                yp = pp.tile([128, 512], F32, tag="pp")
                for c3 in range(8):
                    nc.tensor.matmul(
                        yp[:],
                        outT[:, c3 * N:(c3 + 1) * N],
                        wo_sb[:, c3 * QD + n2 * 512:c3 * QD + (n2 + 1) * 512],
                        start=(c3 == 0), stop=(c3 == 7),
                    )
                nc.vector.tensor_add(
                    y_sb[:, n2 * 512:(n2 + 1) * 512],
                    yp[:],
                    bo_bc[:, n2 * 512:(n2 + 1) * 512],
                )
                eng = nc.sync if n2 == 0 else nc.scalar
                eng.dma_start(
                    yv[:, n2], y_sb[:, n2 * 512:(n2 + 1) * 512])

        pending_proj = None
        kvt_tiles = {0: kvt0_sb}

        def prefetch_kvt(bb):
            t = kvt_pool.tile([128, 4 * M], F16, tag="kvt")
            nc.sync.dma_start(t[:, 0:2 * 2048], kvT_d[bb][:, 0:2 * 2048])
            nc.sync.dma_start(t[:, 2 * 2048:4 * 2048], kvT_d[bb][:, 2 * 2048:4 * 2048])
            kvt_tiles[bb] = t

        for b in range(BPC):
            kvt_sb = kvt_tiles.pop(b)
            kht_list = kht0 if b == 0 else None

            make_vh(kvt_sb)
            # prefetch the next batch's kv during this batch's c-loop
            if b + 1 < BPC:
                prefetch_kvt(b + 1)

            if pending_proj is not None:
                do_outproj(pending_proj)
                pending_proj = None

            # out[n, h*64:+64] staging for this batch + its transposed form
            out_sb = out_pool.tile([128, INNER], F16, tag="out")
            outT_sb = outt_pool.tile([128, INNER], F16, tag="outT")

            # AV for head pair cc: at chunks as stationary, [vh_h | 1] moving;
            # psum col 64 = softmax denominator -> per-partition normalize.
            # One PSUM bank per pair: h0 cols 0:65, h1 cols 65:130, and a
            # f16-bitcast scratch region (130:194) for the out transpose.
            def do_av_mm(avwork, p):
                at_list, cc = avwork
                for hi in range(2):
                    h = 2 * cc + hi
                    for t in range(16):
                        tg, j = t // 2, t % 2
                        nc.tensor.matmul(
                            p[:, hi * 65:hi * 65 + 65],
                            at_list[tg][:, j * 256 + hi * 128:j * 256 + (hi + 1) * 128],
                            vh4[:, t, h, 0:65],
                            start=(t == 0), stop=(t == 15),
                        )

            # evictions issued at the START of the following iteration so
            # they sit ahead of that iteration's kht casts in the DVE queue
            # (the monotonic DVE semaphore then lets the transposes proceed
            # without draining a full iteration of casts first)
            def do_av_evict(cc, p):
                for hi in range(2):
                    h = 2 * cc + hi
                    r32 = r_pool.tile([128, 1], F32, tag="rp32")
                    nc.vector.reciprocal(r32[:], p[:, hi * 65 + 64:hi * 65 + 65])
                    nc.vector.tensor_scalar_mul(
                        out_sb[:, h * 64:(h + 1) * 64],
                        p[:, hi * 65:hi * 65 + 64], r32[:])

            def do_av(avwork, p):
                do_av_mm(avwork, p)
                do_av_evict(avwork[1], p)

            # PE-transpose chunk cc of out_sb into outT_sb, staging through
            # the f16 scratch region of the given pair tile.
            def do_transpose(cc, ptile):
                pt = ptile[:, 130:194].bitcast(F16)
                nc.tensor.transpose(
                    pt, out_sb[:, cc * 128:(cc + 1) * 128], ident[:])
                nc.vector.tensor_copy(outT_sb[:, cc * 128:(cc + 1) * 128], pt)

            # per head pair: khT chunk -> scores -> exp; AV runs one pair
            # behind so the ACT exp latency hides under PE work.
            pending = None
            pending_ev = None
            for c in range(8):
                if pending_ev is not None:
                    do_av_evict(*pending_ev)
                    pending_ev = None
                kht_sb = kht_list[c] if kht_list is not None else make_kht(kvt_sb, c)

                at_tiles = []
                for tg in range(8):
                    sc = scp.tile([128, 512], F32, tag="scp")
                    for j in range(2):
                        t = tg * 2 + j
                        nc.tensor.matmul(
                            sc[:, j * 256:(j + 1) * 256],
                            kht_sb[:, t * 128:(t + 1) * 128],
                            qh2[:, (c * BPC + b) * 256:(c * BPC + b + 1) * 256],
                            start=True, stop=True,
                        )
                    at = attn_pool.tile([128, 512], F16, tag="attn")
                    nc.scalar.activation(at[:], sc[:], AF.Exp)
                    at_tiles.append(at)

                if pending is not None:
                    # transpose BEFORE the AV block: its DVE semaphore
                    # threshold then excludes the newer pair's evictions
                    ptile = avp.tile([128, 194], F32, tag="avp")
                    if pending[1] >= 1:
                        do_transpose(pending[1] - 1, ptile)
                    do_av_mm(pending, ptile)
                    pending_ev = (pending[1], ptile)
                pending = (at_tiles, c)

            if b < BPC - 1:
                if pending_ev is not None:
                    do_av_evict(*pending_ev)
                ptile = avp.tile([128, 194], F32, tag="avp")
                do_av(pending, ptile)
                pending_proj = (b, out_sb, outT_sb)
            else:
                # last batch: quarter the out-projection along Q. Chunks 0-5
                # accumulate while the final AV pair + transposes run, then
                # each quarter's 6-7 finish + bias-add + store pipeline so
                # only one 256-col add and a 128KB store trail the last MM.
                if pending_ev is not None:
                    do_av_evict(*pending_ev)
                y_sb = y_pool.tile([128, QD], F32, tag="yp")
                yps = []
                for q4 in range(4):
                    pool4 = pp if q4 < 2 else scp
                    yp = pool4.tile([128, 256], F32, tag="pp" if q4 < 2 else "scp")
                    for c3 in range(6):
                        nc.tensor.matmul(
                            yp[:],
                            outT_sb[:, c3 * N:(c3 + 1) * N],
                            wo_sb[:, c3 * QD + q4 * 256:c3 * QD + (q4 + 1) * 256],
                            start=(c3 == 0), stop=False,
                        )
                    yps.append(yp)
                ptile = avp.tile([128, 194], F32, tag="avp")
                do_transpose(6, ptile)
                do_av(pending, ptile)
                p8 = avp.tile([128, 194], F32, tag="avp")
                do_transpose(7, p8)
                yv = y_d[b].rearrange("n (h q) -> n h q", h=4)
                for q4 in range(4):
                    for c3 in (6, 7):
                        nc.tensor.matmul(
                            yps[q4][:],
                            outT_sb[:, c3 * N:(c3 + 1) * N],
                            wo_sb[:, c3 * QD + q4 * 256:c3 * QD + (q4 + 1) * 256],
                            start=False, stop=(c3 == 7),
                        )
                    nc.vector.tensor_add(
                        y_sb[:, q4 * 256:(q4 + 1) * 256],
                        yps[q4][:],
                        bo_bc[:, q4 * 256:(q4 + 1) * 256],
                    )
                    eng = nc.sync if q4 % 2 == 0 else nc.scalar
                    eng.dma_start(
                        yv[:, q4], y_sb[:, q4 * 256:(q4 + 1) * 256])
        if pending_proj is not None:
            do_outproj(pending_proj)

    nc.compile()
    return nc


def _get_nc():
    if "nc" not in _cached:
        _cached["nc"] = _build_nc()
    return _cached["nc"]


def kernel(q, kv, Wq, Wk, Wv, Wo, bo):
    from concourse.bass_utils import run_bass_kernel_spmd

    nc = _get_nc()

    # Pre-shuffle everything into [128-partition, free] device layouts.
    # weights (k*128+p, n) -> [p, k, n]
    wq16 = np.ascontiguousarray(
        Wq.reshape(8, 128, INNER).transpose(1, 0, 2).reshape(128, 8 * INNER)
    ).astype(np.float16)
    # Wk -> [p, c, k, ni] so the kht stationary slices are contiguous
    wk16 = np.ascontiguousarray(
        Wk.reshape(4, 128, 8, 128).transpose(1, 2, 0, 3).reshape(128, 4 * INNER)
    ).astype(np.float16)
    wv16 = np.ascontiguousarray(
        Wv.reshape(4, 128, INNER).transpose(1, 0, 2).reshape(128, 4 * INNER)
    ).astype(np.float16)
    wo16 = np.ascontiguousarray(
        Wo.reshape(8, 128, QD).transpose(1, 0, 2).reshape(128, 8 * QD)
    ).astype(np.float16)
    bo32 = np.ascontiguousarray(
        np.broadcast_to(bo.reshape(1, QD), (128, QD)).astype(np.float32))

    scale = D ** -0.5  # 1/8, exact in fp16
    in_maps = []
    for i in range(NCORES):
        bs = slice(i * BPC, (i + 1) * BPC)
        # q (BPC, N, (k p)) -> [p, k, b, n], scale folded in
        qT = np.ascontiguousarray(
            (q[bs] * scale).reshape(BPC, N, 8, 128).transpose(3, 2, 0, 1)
            .reshape(128, 8 * BPC * N)
        ).astype(np.float16)
        # kv (BPC, (mc mi), (k p)) -> [b, p, mc, k, mi]
        kvT = np.ascontiguousarray(
            kv[bs].reshape(BPC, 4, 512, 4, 128).transpose(0, 4, 1, 3, 2)
            .reshape(BPC, 128, 4 * M)
        ).astype(np.float16)
        in_maps.append(
            {"qT": qT, "kvT": kvT, "Wq": wq16, "Wk": wk16, "Wv": wv16,
             "Wo": wo16, "bo": bo32}
        )

    _cached["in_maps"] = in_maps
    res = run_bass_kernel_spmd(nc, in_maps, list(range(NCORES)))
    out = np.concatenate([res.results[i]["y"] for i in range(NCORES)], axis=0)
    return out.astype(np.float32)
